# revision 8
# baseline (speedup 1.0000x reference)
"""Trainium2 Bass kernel for nn_DecoderBlock_90486370992771 (8-core SPMD).

Data-parallel over batch: B=8 -> one batch element per NeuronCore, no
collectives. Per core everything runs in transposed [feature, token]
layout (host pre-transposes x/h and post-transposes the output) so every
matmul's operands arrive in the layout the next matmul needs.

The nonstandard self-attention (per (token, head): softmax over the
causally-masked rank-1 outer product Q[t,h,:] (x) K[t,h,:], attending
over the 64 channels) is computed with a truncated power series:
    num[q] = sum_n (a_q^n / n!) * (L @ (b^n * v))[q]
    den[q] = (q+1) + sum_{n>=1} (a_q^n / n!) * (L @ b^n)[q]
    av[q]  = num[q] / den[q]
where L is the per-head lower-triangular-ones matrix (exact causal mask
as a TensorE matmul), a = Q/sqrt(C), b = K, v = V per (token, head).
max |a*b| ~ 0.9 so 12 Taylor terms give ~1e-10 truncation; the numpy
prototype of this exact scheme matches the fp32 reference to 7e-7.
All matmuls run in float32r (measured 1.5e-4 matmul relerr on HW).
"""
import sys
import math

sys.path.insert(0, "/opt/trn_rl_repo")

import numpy as np

B, S, D = 8, 512, 1024
HID, NH = 1024, 16
C = HID // NH
EPS = 1e-5
NTERMS = 12
NT = D // 128  # 8 feature tiles of 128 partitions
W_NAMES = ["Wq", "Wk", "Wv", "Wo", "Wcq", "Wck", "Wcv", "Wco", "W1", "W2"]
BIAS_NAMES = ["bq_s", "bk", "bv", "bo", "bcq", "bck", "bcv", "bco",
              "b1", "b2", "gamma", "beta"]


def build(nc, debug=False):
    """Emit the full per-core program into `nc` (a bacc.Bacc)."""
    from contextlib import ExitStack
    import concourse.mybir as mybir
    import concourse.tile as tile

    dt = mybir.dt
    f32 = dt.float32
    f32r = dt.float32r
    AF = mybir.ActivationFunctionType
    OP = mybir.AluOpType

    xT_d = nc.dram_tensor("xT", (D, S), f32, kind="ExternalInput")
    hT_d = nc.dram_tensor("hT", (D, S), f32, kind="ExternalInput")
    w_d = {n: nc.dram_tensor(n, (D, HID), f32, kind="ExternalInput")
           for n in W_NAMES}
    b_d = {n: nc.dram_tensor(n, (D,), f32, kind="ExternalInput")
           for n in BIAS_NAMES}
    L2_d = nc.dram_tensor("L2", (128, 128), f32, kind="ExternalInput")
    counts_d = nc.dram_tensor("counts", (128, S), f32, kind="ExternalInput")
    ones_col_d = nc.dram_tensor("ones_col", (128, 1), f32, kind="ExternalInput")
    ones_row_d = nc.dram_tensor("ones_row", (1, 128), f32, kind="ExternalInput")
    outT_d = nc.dram_tensor("outT", (D, S), f32, kind="ExternalOutput")
    dbg_d = {}
    if debug:
        for n in ["avT", "z1", "z2", "ocr"]:
            dbg_d[n] = nc.dram_tensor("dbg_" + n, (D, S), f32,
                                      kind="ExternalOutput")

    with ExitStack() as ctx:
        tc = ctx.enter_context(tile.TileContext(nc))
        big = ctx.enter_context(tc.tile_pool(name="big", bufs=1))
        wk = ctx.enter_context(tc.tile_pool(name="wk", bufs=1))
        sm = ctx.enter_context(tc.tile_pool(name="sm", bufs=1))
        chain = ctx.enter_context(tc.tile_pool(name="chain", bufs=1))
        psp = ctx.enter_context(tc.tile_pool(name="psp", bufs=1, space="PSUM"))

        _ctr = [0]

        def mk(pool, shape, dtype, tag, bufs):
            _ctr[0] += 1
            return pool.tile(list(shape), dtype, tag=tag, bufs=bufs,
                             name=f"{tag}__{_ctr[0]}")

        # shared-slot makers
        def bb(dtype):  # persistent [128, S] activation tiles
            return mk(big, [128, S], dtype, "bb", 50)

        def pp():       # matmul accumulator banks
            return mk(psp, [128, S], f32, "pp", 4)

        def aux(p=128):  # other psum banks
            return mk(psp, [p, S], f32, "aux", 4)

        # ---------------- constants / biases ----------------
        L2_t = mk(big, [128, 128], f32r, "cL2", 1)
        nc.sync.dma_start(L2_t[:], L2_d[:].bitcast(f32r))
        counts_t = mk(big, [128, S], f32, "ccnt", 1)
        nc.sync.dma_start(counts_t[:], counts_d[:])
        ones_col_t = mk(big, [128, 1], f32r, "cones", 1)
        nc.sync.dma_start(ones_col_t[:], ones_col_d[:].bitcast(f32r))
        ones_row_t = mk(big, [1, 128], f32r, "conesr", 1)
        nc.sync.dma_start(ones_row_t[:], ones_row_d[:].bitcast(f32r))

        bcol = {}
        for n in BIAS_NAMES:
            t = mk(big, [128, NT], f32, "bias_" + n, 1)
            nc.sync.dma_start(t[:], b_d[n][:].rearrange("(j p) -> p j", p=128))
            bcol[n] = t

        def bias_slice(name, m):
            return bcol[name][:, m:m + 1]

        eps_col = mk(big, [1, 1], f32, "ceps", 1)
        nc.gpsimd.memset(eps_col[:], EPS)

        # ---------------- inputs ----------------
        def load_T(dram):
            ts = []
            for m in range(NT):
                t = bb(f32r)
                nc.sync.dma_start(
                    t[:], dram[m * 128:(m + 1) * 128, :].bitcast(f32r))
                ts.append(t)
            return ts

        xT = load_T(xT_d)
        hT = load_T(hT_d)

        # ---------------- generic projection ----------------
        def proj(wname, rhs_tiles, consume):
            """out[m] = consume(m, sum_k W[kblk, mblk].T @ rhs[k]) for 8 m."""
            outs = []
            for m in range(NT):
                psum = pp()
                for k in range(NT):
                    wt = mk(wk, [128, 128], f32r, "w", 8)
                    nc.sync.dma_start(
                        wt[:],
                        w_d[wname][k * 128:(k + 1) * 128,
                                   m * 128:(m + 1) * 128].bitcast(f32r))
                    nc.tensor.matmul(psum[:], wt[:], rhs_tiles[k][:],
                                     start=(k == 0), stop=(k == NT - 1))
                outs.append(consume(m, psum))
            return outs

        def copy_out(dtype, scale, bias_name):
            def f(m, psum):
                t = bb(dtype)
                nc.scalar.activation(t[:], psum[:], AF.Identity,
                                     bias=bias_slice(bias_name, m), scale=scale)
                return t
            return f

        def resid_out(bias_name, other_tiles, dtype=f32r):
            def f(m, psum):
                t = bb(dtype)
                nc.vector.scalar_tensor_tensor(
                    t[:], psum[:], bias_slice(bias_name, m), other_tiles[m][:],
                    op0=OP.add, op1=OP.add)
                return t
            return f

        # ---------------- layernorm (transposed layout) ----------------
        def ln_row(dtype=f32):
            return mk(sm, [1, S], dtype, "ln_row", 4)

        def layer_norm(in_tiles, out_dtype=f32r):
            mu_ps = aux(1)
            s2_ps = aux(1)
            for m in range(NT):
                nc.tensor.matmul(mu_ps[:], ones_col_t[:], in_tiles[m][:],
                                 start=(m == 0), stop=(m == NT - 1))
            for m in range(NT):
                sq = mk(sm, [128, S], f32r, "ln_sq", 2)
                nc.scalar.activation(sq[:], in_tiles[m][:], AF.Square)
                nc.tensor.matmul(s2_ps[:], ones_col_t[:], sq[:],
                                 start=(m == 0), stop=(m == NT - 1))
            mu_row = ln_row(f32r)
            s2_row = ln_row()
            nc.scalar.activation(mu_row[:], mu_ps[:], AF.Copy, scale=1.0 / D)
            nc.scalar.activation(s2_row[:], s2_ps[:], AF.Copy, scale=1.0 / D)
            var_row = ln_row()
            # var = (mu * -1) * mu + s2
            nc.vector.scalar_tensor_tensor(
                var_row[:], mu_row[:], -1.0, mu_row[:],
                op0=OP.mult, op1=OP.mult)
            nc.vector.tensor_add(var_row[:], var_row[:], s2_row[:])
            lnv = ln_row()
            nc.scalar.activation(lnv[:], var_row[:], AF.Ln, bias=eps_col[:])
            rstd_row = ln_row(f32r)
            nc.scalar.activation(rstd_row[:], lnv[:], AF.Exp, scale=-0.5)
            mu_rep = aux()
            rs_rep = aux()
            nc.tensor.matmul(mu_rep[:], ones_row_t[:], mu_row[:],
                             start=True, stop=True)
            nc.tensor.matmul(rs_rep[:], ones_row_t[:], rstd_row[:],
                             start=True, stop=True)
            rs_rep_sb = mk(sm, [128, S], f32, "ln_rsrep", 2)
            nc.vector.tensor_copy(rs_rep_sb[:], rs_rep[:])
            outs = []
            for m in range(NT):
                diff = mk(sm, [128, S], f32, "ln_tmp", 3)
                nc.vector.tensor_sub(diff[:], in_tiles[m][:], mu_rep[:])
                g = mk(sm, [128, S], f32, "ln_tmp", 3)
                nc.vector.scalar_tensor_tensor(
                    g[:], diff[:], bias_slice("gamma", m), rs_rep_sb[:],
                    op0=OP.mult, op1=OP.mult)
                o = bb(out_dtype)
                nc.scalar.activation(o[:], g[:], AF.Identity,
                                     bias=bias_slice("beta", m))
                outs.append(o)
            return outs

        # ================= stage 1: self attention =================
        A1 = proj("Wq", xT, copy_out(f32r, 1.0 / math.sqrt(C), "bq_s"))
        P1 = proj("Wk", xT, copy_out(f32r, 1.0, "bk"))
        G0 = proj("Wv", xT, copy_out(f32r, 1.0, "bv"))

        def ch(dtype, tag, bufs=2):
            return mk(chain, [128, S], dtype, tag, bufs)

        avT = []
        for i in range(NT):
            num = ch(f32, "num")
            den = ch(f32, "den")
            t_ps = aux()
            nc.tensor.matmul(t_ps[:], L2_t[:], G0[i][:], start=True, stop=True)
            nc.scalar.activation(num[:], t_ps[:], AF.Copy)
            An_prev, Pn_prev, Gn_prev = None, None, None
            for n in range(1, NTERMS):
                if n == 1:
                    An, Pn = A1[i], P1[i]
                    Gn = ch(f32r, "Gn")
                    nc.vector.tensor_mul(Gn[:], G0[i][:], P1[i][:])
                else:
                    An = ch(f32, "An")
                    nc.vector.scalar_tensor_tensor(
                        An[:], An_prev[:], 1.0 / n, A1[i][:],
                        op0=OP.mult, op1=OP.mult)
                    Pn = ch(f32r, "Pn")
                    nc.vector.tensor_mul(Pn[:], Pn_prev[:], P1[i][:])
                    Gn = ch(f32r, "Gn")
                    nc.vector.tensor_mul(Gn[:], Gn_prev[:], P1[i][:])
                t_ps = aux()
                w_ps = aux()
                nc.tensor.matmul(t_ps[:], L2_t[:], Gn[:], start=True, stop=True)
                nc.tensor.matmul(w_ps[:], L2_t[:], Pn[:], start=True, stop=True)
                tmp = ch(f32, "tmp", 2)
                nc.vector.tensor_mul(tmp[:], An[:], t_ps[:])
                nc.vector.tensor_add(num[:], num[:], tmp[:])
                if n == 1:
                    nc.vector.tensor_mul(den[:], An[:], w_ps[:])
                else:
                    tmp2 = ch(f32, "tmp", 2)
                    nc.vector.tensor_mul(tmp2[:], An[:], w_ps[:])
                    nc.vector.tensor_add(den[:], den[:], tmp2[:])
                An_prev, Pn_prev, Gn_prev = An, Pn, Gn
            nc.vector.tensor_add(den[:], den[:], counts_t[:])
            lg = ch(f32, "tmp", 2)
            nc.scalar.activation(lg[:], den[:], AF.Ln)
            rec = ch(f32, "tmp", 2)
            nc.scalar.activation(rec[:], lg[:], AF.Exp, scale=-1.0)
            av = bb(f32r)
            nc.vector.tensor_mul(av[:], num[:], rec[:])
            avT.append(av)

        r1 = proj("Wo", avT, resid_out("bo", xT))
        z1 = layer_norm(r1)

        # ================= stage 2: cross attention =================
        QcT = proj("Wcq", z1, copy_out(f32r, 1.0, "bcq"))
        KcT = proj("Wck", hT, copy_out(f32r, 1.0, "bck"))

        # bcv replicated across partitions (it varies along the free dim here)
        bcv_row = mk(sm, [1, HID], f32r, "bcv_row", 1)
        nc.sync.dma_start(
            bcv_row[:],
            b_d["bcv"][:].rearrange("(o d) -> o d", o=1).bitcast(f32r))
        bcv_rep = []
        for half in range(2):
            rep_ps = aux()
            nc.tensor.matmul(rep_ps[:], ones_row_t[:],
                             bcv_row[:, half * S:(half + 1) * S],
                             start=True, stop=True)
            rep_sb = mk(sm, [128, S], f32, "rep_sb", 2)
            nc.vector.tensor_copy(rep_sb[:], rep_ps[:])
            bcv_rep.append(rep_sb)
        # Vc natural [S, HID] in 4 row-tiles of [128, HID]
        Vc = [mk(big, [128, HID], f32r, "vc", 4) for _ in range(4)]
        for half in range(2):
            vps = [pp() for _ in range(4)]
            for k in range(NT):
                wt = mk(wk, [128, S], f32r, "wv", 3)
                nc.sync.dma_start(
                    wt[:],
                    w_d["Wcv"][k * 128:(k + 1) * 128,
                               half * S:(half + 1) * S].bitcast(f32r))
                for kt in range(4):
                    nc.tensor.matmul(
                        vps[kt][:], hT[k][:, kt * 128:(kt + 1) * 128], wt[:],
                        start=(k == 0), stop=(k == NT - 1))
            for kt in range(4):
                nc.vector.tensor_add(Vc[kt][:, half * S:(half + 1) * S],
                                     vps[kt][:], bcv_rep[half][:])

        # per-head cross attention; softmax normalization folded into ES
        o_cr = [bb(f32r) for _ in range(NT)]
        for hd in range(NH):
            i, r0 = hd // 2, (hd % 2) * 64
            es_tiles = []
            for kt in range(4):
                s_ps = pp()
                nc.tensor.matmul(
                    s_ps[:], KcT[i][r0:r0 + 64, kt * 128:(kt + 1) * 128],
                    QcT[i][r0:r0 + 64, :], start=True, stop=True)
                es = mk(sm, [128, S], f32r, "cr_es", 5)
                nc.scalar.activation(es[:], s_ps[:], AF.Exp,
                                     scale=1.0 / math.sqrt(HID))
                es_tiles.append(es)
            d_ps = aux(1)
            for kt in range(4):
                nc.tensor.matmul(d_ps[:], ones_col_t[:], es_tiles[kt][:],
                                 start=(kt == 0), stop=(kt == 3))
            lg_row = mk(sm, [1, S], f32, "cr_row", 2)
            nc.scalar.activation(lg_row[:], d_ps[:], AF.Ln)
            rec_row = mk(sm, [1, S], f32r, "cr_row", 2)
            nc.scalar.activation(rec_row[:], lg_row[:], AF.Exp, scale=-1.0)
            rep_ps = aux()
            nc.tensor.matmul(rep_ps[:], ones_row_t[:], rec_row[:],
                             start=True, stop=True)
            for kt in range(4):
                nc.vector.tensor_mul(es_tiles[kt][:], es_tiles[kt][:],
                                     rep_ps[:])
            o_ps = aux(64)
            for kt in range(4):
                nc.tensor.matmul(o_ps[:], Vc[kt][:, hd * 64:(hd + 1) * 64],
                                 es_tiles[kt][:], start=(kt == 0),
                                 stop=(kt == 3))
            nc.scalar.activation(o_cr[i][r0:r0 + 64, :], o_ps[:], AF.Copy)

        r2 = proj("Wco", o_cr, resid_out("bco", z1))
        z2 = layer_norm(r2)

        # ================= stage 3: FFN =================
        u = proj("W1", z2, copy_out(f32r, 1.0, "b1"))
        r3 = proj("W2", u, resid_out("b2", z2))
        z3 = layer_norm(r3, out_dtype=f32)

        for m in range(NT):
            nc.sync.dma_start(outT_d[m * 128:(m + 1) * 128, :], z3[m][:])
        if debug:
            dbg_src = {"avT": avT, "z1": z1, "z2": z2, "ocr": o_cr}
            for n, tiles in dbg_src.items():
                for m in range(NT):
                    nc.sync.dma_start(
                        dbg_d[n][m * 128:(m + 1) * 128, :],
                        tiles[m][:].bitcast(f32))


def make_consts():
    L = np.zeros((128, 128), np.float32)
    for k in range(128):
        for q in range(128):
            if k // 64 == q // 64 and (k % 64) <= (q % 64):
                L[k, q] = 1.0
    counts = np.tile((np.arange(128, dtype=np.float32) % 64) + 1.0,
                     (S, 1)).T.copy()
    return {
        "L2": L,
        "counts": np.ascontiguousarray(counts),
        "ones_col": np.ones((128, 1), np.float32),
        "ones_row": np.ones((1, 128), np.float32),
    }


def make_in_maps(inputs):
    x = np.asarray(inputs["x"], np.float32)
    h = np.asarray(inputs["h"], np.float32)
    consts = make_consts()
    base = {n: np.ascontiguousarray(np.asarray(inputs[n], np.float32))
            for n in W_NAMES}
    biases = {"bq_s": np.asarray(inputs["bq"], np.float32) / math.sqrt(C)}
    for n in BIAS_NAMES[1:]:
        biases[n] = inputs[n]
    biases = {k: np.ascontiguousarray(np.asarray(v, np.float32))
              for k, v in biases.items()}
    in_maps = []
    for b in range(B):
        m = {"xT": np.ascontiguousarray(x[b].T),
             "hT": np.ascontiguousarray(h[b].T)}
        m.update(base)
        m.update(biases)
        m.update(consts)
        in_maps.append(m)
    return in_maps


_CACHE = {}


def get_program(debug=False):
    key = ("prog", debug)
    if key not in _CACHE:
        import concourse.bacc as bacc
        nc = bacc.Bacc(trn_type="TRN2")
        build(nc, debug=debug)
        nc.finalize()
        _CACHE[key] = nc
    return _CACHE[key]


def kernel(**inputs):
    from concourse.bass_utils import run_bass_kernel_spmd

    nc = get_program()
    in_maps = make_in_maps(inputs)
    res = run_bass_kernel_spmd(nc, in_maps, list(range(8)))
    out = np.stack([np.asarray(res.results[b]["outT"]).T for b in range(B)])
    return out.astype(np.float32)


if __name__ == "__main__":
    nc = get_program()
    print("built:", len(nc.inst_map), "instructions")


# revision 15
# speedup vs baseline: 1.3322x; 1.3322x over previous
"""Trainium2 Bass kernel for nn_DecoderBlock_90486370992771 (8-core SPMD).

Data-parallel over batch: B=8 -> one batch element per NeuronCore, no
collectives. Per core everything runs in transposed [feature, token]
layout (host pre-transposes x/h and post-transposes the output) so every
matmul's operands arrive in the layout the next matmul needs.

The nonstandard self-attention (per (token, head): softmax over the
causally-masked rank-1 outer product Q[t,h,:] (x) K[t,h,:], attending
over the 64 channels) is computed with a truncated power series:
    num[q] = sum_n (a_q^n / n!) * (L @ (b^n * v))[q]
    den[q] = (q+1) + sum_{n>=1} (a_q^n / n!) * (L @ b^n)[q]
    av[q]  = num[q] / den[q]
where L is the per-head lower-triangular-ones matrix (exact causal mask
as a TensorE matmul), a = Q/sqrt(C), b = K, v = V per (token, head).
max |a*b| ~ 0.9 so 12 Taylor terms give ~1e-10 truncation; the numpy
prototype of this exact scheme matches the fp32 reference to 7e-7.
All matmuls run in float32r (measured 1.5e-4 matmul relerr on HW).
"""
import sys
import math

sys.path.insert(0, "/opt/trn_rl_repo")

import numpy as np

B, S, D = 8, 512, 1024
HID, NH = 1024, 16
C = HID // NH
EPS = 1e-5
NTERMS = 10
NT = D // 128  # 8 feature tiles of 128 partitions
W_NAMES = ["Wq", "Wk", "Wv", "Wo", "Wcq", "Wck", "Wcv", "Wco", "W1", "W2"]
BIAS_NAMES = ["bq_s", "bk", "bv", "bo", "bcq", "bck", "bcv", "bco",
              "b1", "b2", "gamma", "beta"]


def build(nc, debug=False):
    """Emit the full per-core program into `nc` (a bacc.Bacc)."""
    from contextlib import ExitStack
    import concourse.mybir as mybir
    import concourse.tile as tile

    dt = mybir.dt
    f32 = dt.float32
    f32r = dt.float32r
    AF = mybir.ActivationFunctionType
    OP = mybir.AluOpType

    xT_d = nc.dram_tensor("xT", (D, S), f32, kind="ExternalInput")
    hT_d = nc.dram_tensor("hT", (D, S), f32, kind="ExternalInput")
    w_d = {n: nc.dram_tensor(n, (D, HID), f32, kind="ExternalInput")
           for n in W_NAMES}
    b_d = {n: nc.dram_tensor(n, (D,), f32, kind="ExternalInput")
           for n in BIAS_NAMES}
    L2_d = nc.dram_tensor("L2", (128, 128), f32, kind="ExternalInput")
    counts_d = nc.dram_tensor("counts", (128, S), f32, kind="ExternalInput")
    ones_col_d = nc.dram_tensor("ones_col", (128, 1), f32, kind="ExternalInput")
    ones_row_d = nc.dram_tensor("ones_row", (1, 128), f32, kind="ExternalInput")
    outT_d = nc.dram_tensor("outT", (D, S), f32, kind="ExternalOutput")
    dbg_d = {}
    if debug:
        for n in ["avT", "z1", "z2", "ocr"]:
            dbg_d[n] = nc.dram_tensor("dbg_" + n, (D, S), f32,
                                      kind="ExternalOutput")

    with ExitStack() as ctx:
        tc = ctx.enter_context(tile.TileContext(nc))
        big = ctx.enter_context(tc.tile_pool(name="big", bufs=1))
        wk = ctx.enter_context(tc.tile_pool(name="wk", bufs=1))
        sm = ctx.enter_context(tc.tile_pool(name="sm", bufs=1))
        chain = ctx.enter_context(tc.tile_pool(name="chain", bufs=1))
        psp = ctx.enter_context(tc.tile_pool(name="psp", bufs=1, space="PSUM"))

        _ctr = [0]

        def mk(pool, shape, dtype, tag, bufs):
            _ctr[0] += 1
            return pool.tile(list(shape), dtype, tag=tag, bufs=bufs,
                             name=f"{tag}__{_ctr[0]}")

        # shared-slot makers
        def bb(dtype):  # persistent [128, S] activation tiles
            return mk(big, [128, S], dtype, "bb", 50)

        def pp():       # matmul accumulator banks
            return mk(psp, [128, S], f32, "pp", 4)

        def aux(p=128):  # other psum banks
            return mk(psp, [p, S], f32, "aux", 4)

        # ---------------- constants / biases ----------------
        L2_t = mk(big, [128, 128], f32r, "cL2", 1)
        nc.sync.dma_start(L2_t[:], L2_d[:].bitcast(f32r))
        counts_t = mk(big, [128, S], f32, "ccnt", 1)
        nc.sync.dma_start(counts_t[:], counts_d[:])
        ones_col_t = mk(big, [128, 1], f32r, "cones", 1)
        nc.sync.dma_start(ones_col_t[:], ones_col_d[:].bitcast(f32r))
        ones_row_t = mk(big, [1, 128], f32r, "conesr", 1)
        nc.sync.dma_start(ones_row_t[:], ones_row_d[:].bitcast(f32r))

        bcol = {}
        for n in BIAS_NAMES:
            t = mk(big, [128, NT], f32, "bias_" + n, 1)
            nc.sync.dma_start(t[:], b_d[n][:].rearrange("(j p) -> p j", p=128))
            bcol[n] = t

        def bias_slice(name, m):
            return bcol[name][:, m:m + 1]

        eps_col = mk(big, [1, 1], f32, "ceps", 1)
        nc.gpsimd.memset(eps_col[:], EPS)

        # ---------------- inputs ----------------
        def load_T(dram):
            ts = []
            for m in range(NT):
                t = bb(f32r)
                nc.sync.dma_start(
                    t[:], dram[m * 128:(m + 1) * 128, :].bitcast(f32r))
                ts.append(t)
            return ts

        xT = load_T(xT_d)
        hT = load_T(hT_d)

        # ---------------- generic projection ----------------
        def wrow_load(wname, half):
            """DMA the [1024, 512] half of W as 8 [128, 512] row tiles."""
            ts = []
            for k in range(NT):
                wt = mk(wk, [128, S], f32r, "w", 12)
                nc.sync.dma_start(
                    wt[:],
                    w_d[wname][k * 128:(k + 1) * 128,
                               half * S:(half + 1) * S].bitcast(f32r))
                ts.append(wt)
            return ts

        def proj(wname, rhs_tiles, consume):
            """out[m] = consume(m, sum_k W[kblk, mblk].T @ rhs[k]) for 8 m."""
            outs = []
            for half in range(2):
                wrows = wrow_load(wname, half)
                for mm in range(4):
                    m = half * 4 + mm
                    psum = pp()
                    for k in range(NT):
                        nc.tensor.matmul(
                            psum[:], wrows[k][:, mm * 128:(mm + 1) * 128],
                            rhs_tiles[k][:], start=(k == 0),
                            stop=(k == NT - 1))
                    outs.append(consume(m, psum))
            return outs

        def copy_out(dtype, scale, bias_name):
            def f(m, psum):
                t = bb(dtype)
                nc.scalar.activation(t[:], psum[:], AF.Identity,
                                     bias=bias_slice(bias_name, m), scale=scale)
                return t
            return f

        def resid_out(bias_name, other_tiles, dtype=f32r):
            def f(m, psum):
                t = bb(dtype)
                nc.vector.scalar_tensor_tensor(
                    t[:], psum[:], bias_slice(bias_name, m), other_tiles[m][:],
                    op0=OP.add, op1=OP.add)
                return t
            return f

        # ---------------- layernorm (transposed layout) ----------------
        def ln_row(dtype=f32):
            return mk(sm, [1, S], dtype, "ln_row", 3)

        def layer_norm(in_tiles, out_dtype=f32r):
            mu_ps = aux(1)
            s2_ps = aux(1)
            for m in range(NT):
                nc.tensor.matmul(mu_ps[:], ones_col_t[:], in_tiles[m][:],
                                 start=(m == 0), stop=(m == NT - 1))
            for m in range(NT):
                sq = mk(sm, [128, S], f32r, "ln_sq", 2)
                nc.scalar.activation(sq[:], in_tiles[m][:], AF.Square)
                nc.tensor.matmul(s2_ps[:], ones_col_t[:], sq[:],
                                 start=(m == 0), stop=(m == NT - 1))
            mu_row = ln_row(f32r)
            s2_row = ln_row()
            nc.scalar.activation(mu_row[:], mu_ps[:], AF.Copy, scale=1.0 / D)
            nc.scalar.activation(s2_row[:], s2_ps[:], AF.Copy, scale=1.0 / D)
            var_row = ln_row()
            # var = (mu * -1) * mu + s2
            nc.vector.scalar_tensor_tensor(
                var_row[:], mu_row[:], -1.0, mu_row[:],
                op0=OP.mult, op1=OP.mult)
            nc.vector.tensor_add(var_row[:], var_row[:], s2_row[:])
            lnv = ln_row()
            nc.scalar.activation(lnv[:], var_row[:], AF.Ln, bias=eps_col[:])
            rstd_row = ln_row(f32r)
            nc.scalar.activation(rstd_row[:], lnv[:], AF.Exp, scale=-0.5)
            mu_rep = aux()
            rs_rep = aux()
            nc.tensor.matmul(mu_rep[:], ones_row_t[:], mu_row[:],
                             start=True, stop=True)
            nc.tensor.matmul(rs_rep[:], ones_row_t[:], rstd_row[:],
                             start=True, stop=True)
            rs_rep_sb = mk(sm, [128, S], f32, "ln_rsrep", 2)
            nc.vector.tensor_copy(rs_rep_sb[:], rs_rep[:])
            outs = []
            for m in range(NT):
                diff = mk(sm, [128, S], f32, "ln_tmp", 2)
                nc.vector.tensor_sub(diff[:], in_tiles[m][:], mu_rep[:])
                g = mk(sm, [128, S], f32, "ln_tmp", 2)
                nc.vector.scalar_tensor_tensor(
                    g[:], diff[:], bias_slice("gamma", m), rs_rep_sb[:],
                    op0=OP.mult, op1=OP.mult)
                o = bb(out_dtype)
                nc.scalar.activation(o[:], g[:], AF.Identity,
                                     bias=bias_slice("beta", m))
                outs.append(o)
            return outs

        # ================= stage 1: self attention =================
        # QKV projections interleaved per m-tile so the tile-0 series can
        # start while PE continues projecting tiles 1..7 (phase overlap).
        A1, P1, G0 = [], [], []
        qkv_spec = [
            ("Wq", A1, copy_out(f32r, 1.0 / math.sqrt(C), "bq_s")),
            ("Wk", P1, copy_out(f32r, 1.0, "bk")),
            ("Wv", G0, copy_out(f32r, 1.0, "bv")),
        ]
        for half in range(2):
            for wname, lst, consume in qkv_spec:
                wrows = wrow_load(wname, half)
                for mm in range(4):
                    m = half * 4 + mm
                    psum = pp()
                    for k in range(NT):
                        nc.tensor.matmul(
                            psum[:], wrows[k][:, mm * 128:(mm + 1) * 128],
                            xT[k][:], start=(k == 0), stop=(k == NT - 1))
                    lst.append(consume(m, psum))

        def ch(dtype, tag, bufs=2):
            return mk(chain, [128, S], dtype, tag, bufs)

        KcT = proj("Wck", hT, copy_out(f32r, 1.0, "bck"))
        # bcv replicated across partitions (it varies along the free dim here)
        bcv_row = mk(sm, [1, HID], f32r, "bcv_row", 1)
        nc.sync.dma_start(
            bcv_row[:],
            b_d["bcv"][:].rearrange("(o d) -> o d", o=1).bitcast(f32r))
        bcv_rep = []
        for half in range(2):
            rep_ps = aux()
            nc.tensor.matmul(rep_ps[:], ones_row_t[:],
                             bcv_row[:, half * S:(half + 1) * S],
                             start=True, stop=True)
            rep_sb = mk(sm, [128, S], f32, "rep_sb", 2)
            nc.vector.tensor_copy(rep_sb[:], rep_ps[:])
            bcv_rep.append(rep_sb)
        # Vc natural [S, HID] in 4 row-tiles of [128, HID]
        Vc = [mk(big, [128, HID], f32r, "vc", 4) for _ in range(4)]
        for half in range(2):
            vps = [pp() for _ in range(4)]
            for k in range(NT):
                wt = mk(wk, [128, S], f32r, "w", 12)
                nc.sync.dma_start(
                    wt[:],
                    w_d["Wcv"][k * 128:(k + 1) * 128,
                               half * S:(half + 1) * S].bitcast(f32r))
                for kt in range(4):
                    nc.tensor.matmul(
                        vps[kt][:], hT[k][:, kt * 128:(kt + 1) * 128], wt[:],
                        start=(k == 0), stop=(k == NT - 1))
            for kt in range(4):
                nc.vector.tensor_add(Vc[kt][:, half * S:(half + 1) * S],
                                     vps[kt][:], bcv_rep[half][:])

        avT = []
        for i in range(NT):
            num = ch(f32, "num")
            den = ch(f32, "den")
            t_ps = aux()
            nc.tensor.matmul(t_ps[:], L2_t[:], G0[i][:], start=True, stop=True)
            nc.scalar.activation(num[:], t_ps[:], AF.Copy)
            An_prev, Pn_prev, Gn_prev = None, None, None
            for n in range(1, NTERMS):
                if n == 1:
                    An, Pn = A1[i], P1[i]
                    Gn = ch(f32r, "Gn")
                    nc.vector.tensor_mul(Gn[:], G0[i][:], P1[i][:])
                else:
                    An = ch(f32, "An")
                    nc.vector.scalar_tensor_tensor(
                        An[:], An_prev[:], 1.0 / n, A1[i][:],
                        op0=OP.mult, op1=OP.mult)
                    Pn = ch(f32r, "Pn")
                    nc.gpsimd.tensor_mul(Pn[:], Pn_prev[:], P1[i][:])
                    Gn = ch(f32r, "Gn")
                    nc.vector.tensor_mul(Gn[:], Gn_prev[:], P1[i][:])
                t_ps = aux()
                w_ps = aux()
                nc.tensor.matmul(t_ps[:], L2_t[:], Gn[:], start=True, stop=True)
                nc.tensor.matmul(w_ps[:], L2_t[:], Pn[:], start=True, stop=True)
                tmp = ch(f32, "tmp", 2)
                nc.vector.tensor_mul(tmp[:], An[:], t_ps[:])
                nc.gpsimd.tensor_add(num[:], num[:], tmp[:])
                if n == 1:
                    nc.vector.tensor_mul(den[:], An[:], w_ps[:])
                else:
                    tmp2 = ch(f32, "tmp2", 1)
                    nc.vector.tensor_mul(tmp2[:], An[:], w_ps[:])
                    nc.gpsimd.tensor_add(den[:], den[:], tmp2[:])
                An_prev, Pn_prev, Gn_prev = An, Pn, Gn
            nc.gpsimd.tensor_add(den[:], den[:], counts_t[:])
            lg = ch(f32, "tmp", 2)
            nc.scalar.activation(lg[:], den[:], AF.Ln)
            rec = ch(f32, "tmp", 2)
            nc.scalar.activation(rec[:], lg[:], AF.Exp, scale=-1.0)
            av = bb(f32r)
            nc.vector.tensor_mul(av[:], num[:], rec[:])
            avT.append(av)

        r1 = proj("Wo", avT, resid_out("bo", xT))
        z1 = layer_norm(r1)

        # ================= stage 2: cross attention =================
        QcT = proj("Wcq", z1, copy_out(f32r, 1.0, "bcq"))

        # per-head cross attention; softmax normalization folded into ES
        o_cr = [bb(f32r) for _ in range(NT)]
        for hd in range(NH):
            i, r0 = hd // 2, (hd % 2) * 64
            es_tiles = []
            for kt in range(4):
                s_ps = pp()
                nc.tensor.matmul(
                    s_ps[:], KcT[i][r0:r0 + 64, kt * 128:(kt + 1) * 128],
                    QcT[i][r0:r0 + 64, :], start=True, stop=True)
                es = mk(sm, [128, S], f32r, "cr_es", 4)
                nc.scalar.activation(es[:], s_ps[:], AF.Exp,
                                     scale=1.0 / math.sqrt(HID))
                es_tiles.append(es)
            d_ps = aux(1)
            for kt in range(4):
                nc.tensor.matmul(d_ps[:], ones_col_t[:], es_tiles[kt][:],
                                 start=(kt == 0), stop=(kt == 3))
            lg_row = mk(sm, [1, S], f32, "cr_row", 2)
            nc.scalar.activation(lg_row[:], d_ps[:], AF.Ln)
            rec_row = mk(sm, [1, S], f32r, "cr_row", 2)
            nc.scalar.activation(rec_row[:], lg_row[:], AF.Exp, scale=-1.0)
            rep_ps = aux()
            nc.tensor.matmul(rep_ps[:], ones_row_t[:], rec_row[:],
                             start=True, stop=True)
            for kt in range(4):
                nc.vector.tensor_mul(es_tiles[kt][:], es_tiles[kt][:],
                                     rep_ps[:])
            o_ps = aux(64)
            for kt in range(4):
                nc.tensor.matmul(o_ps[:], Vc[kt][:, hd * 64:(hd + 1) * 64],
                                 es_tiles[kt][:], start=(kt == 0),
                                 stop=(kt == 3))
            nc.scalar.activation(o_cr[i][r0:r0 + 64, :], o_ps[:], AF.Copy)

        r2 = proj("Wco", o_cr, resid_out("bco", z1))
        z2 = layer_norm(r2)

        # ================= stage 3: FFN =================
        u = proj("W1", z2, copy_out(f32r, 1.0, "b1"))
        r3 = proj("W2", u, resid_out("b2", z2))
        z3 = layer_norm(r3, out_dtype=f32)

        for m in range(NT):
            nc.sync.dma_start(outT_d[m * 128:(m + 1) * 128, :], z3[m][:])
        if debug:
            dbg_src = {"avT": avT, "z1": z1, "z2": z2, "ocr": o_cr}
            for n, tiles in dbg_src.items():
                for m in range(NT):
                    nc.sync.dma_start(
                        dbg_d[n][m * 128:(m + 1) * 128, :],
                        tiles[m][:].bitcast(f32))


def make_consts():
    L = np.zeros((128, 128), np.float32)
    for k in range(128):
        for q in range(128):
            if k // 64 == q // 64 and (k % 64) <= (q % 64):
                L[k, q] = 1.0
    counts = np.tile((np.arange(128, dtype=np.float32) % 64) + 1.0,
                     (S, 1)).T.copy()
    return {
        "L2": L,
        "counts": np.ascontiguousarray(counts),
        "ones_col": np.ones((128, 1), np.float32),
        "ones_row": np.ones((1, 128), np.float32),
    }


def make_in_maps(inputs):
    x = np.asarray(inputs["x"], np.float32)
    h = np.asarray(inputs["h"], np.float32)
    consts = make_consts()
    base = {n: np.ascontiguousarray(np.asarray(inputs[n], np.float32))
            for n in W_NAMES}
    biases = {"bq_s": np.asarray(inputs["bq"], np.float32) / math.sqrt(C)}
    for n in BIAS_NAMES[1:]:
        biases[n] = inputs[n]
    biases = {k: np.ascontiguousarray(np.asarray(v, np.float32))
              for k, v in biases.items()}
    in_maps = []
    for b in range(B):
        m = {"xT": np.ascontiguousarray(x[b].T),
             "hT": np.ascontiguousarray(h[b].T)}
        m.update(base)
        m.update(biases)
        m.update(consts)
        in_maps.append(m)
    return in_maps


_CACHE = {}


def get_program(debug=False):
    key = ("prog", debug)
    if key not in _CACHE:
        import concourse.bacc as bacc
        nc = bacc.Bacc(trn_type="TRN2")
        build(nc, debug=debug)
        nc.finalize()
        _CACHE[key] = nc
    return _CACHE[key]


def kernel(**inputs):
    from concourse.bass_utils import run_bass_kernel_spmd

    nc = get_program()
    in_maps = make_in_maps(inputs)
    res = run_bass_kernel_spmd(nc, in_maps, list(range(8)))
    out = np.stack([np.asarray(res.results[b]["outT"]).T for b in range(B)])
    return out.astype(np.float32)


if __name__ == "__main__":
    nc = get_program()
    print("built:", len(nc.inst_map), "instructions")


# revision 16
# speedup vs baseline: 1.3557x; 1.0176x over previous
"""Trainium2 Bass kernel for nn_DecoderBlock_90486370992771 (8-core SPMD).

Data-parallel over batch: B=8 -> one batch element per NeuronCore, no
collectives. Per core everything runs in transposed [feature, token]
layout (host pre-transposes x/h and post-transposes the output) so every
matmul's operands arrive in the layout the next matmul needs.

The nonstandard self-attention (per (token, head): softmax over the
causally-masked rank-1 outer product Q[t,h,:] (x) K[t,h,:], attending
over the 64 channels) is computed with a truncated power series:
    num[q] = sum_n (a_q^n / n!) * (L @ (b^n * v))[q]
    den[q] = (q+1) + sum_{n>=1} (a_q^n / n!) * (L @ b^n)[q]
    av[q]  = num[q] / den[q]
where L is the per-head lower-triangular-ones matrix (exact causal mask
as a TensorE matmul), a = Q/sqrt(C), b = K, v = V per (token, head).
max |a*b| ~ 0.9 so 12 Taylor terms give ~1e-10 truncation; the numpy
prototype of this exact scheme matches the fp32 reference to 7e-7.
All matmuls run in float32r (measured 1.5e-4 matmul relerr on HW).
"""
import sys
import math

sys.path.insert(0, "/opt/trn_rl_repo")

import numpy as np

B, S, D = 8, 512, 1024
HID, NH = 1024, 16
C = HID // NH
EPS = 1e-5
NTERMS = 9
NT = D // 128  # 8 feature tiles of 128 partitions
W_NAMES = ["Wq", "Wk", "Wv", "Wo", "Wcq", "Wck", "Wcv", "Wco", "W1", "W2"]
BIAS_NAMES = ["bq_s", "bk", "bv", "bo", "bcq", "bck", "bcv", "bco",
              "b1", "b2", "gamma", "beta"]


def build(nc, debug=False):
    """Emit the full per-core program into `nc` (a bacc.Bacc)."""
    from contextlib import ExitStack
    import concourse.mybir as mybir
    import concourse.tile as tile

    dt = mybir.dt
    f32 = dt.float32
    f32r = dt.float32r
    AF = mybir.ActivationFunctionType
    OP = mybir.AluOpType

    xT_d = nc.dram_tensor("xT", (D, S), f32, kind="ExternalInput")
    hT_d = nc.dram_tensor("hT", (D, S), f32, kind="ExternalInput")
    w_d = {n: nc.dram_tensor(n, (D, HID), f32, kind="ExternalInput")
           for n in W_NAMES}
    b_d = {n: nc.dram_tensor(n, (D,), f32, kind="ExternalInput")
           for n in BIAS_NAMES}
    L2_d = nc.dram_tensor("L2", (128, 128), f32, kind="ExternalInput")
    counts_d = nc.dram_tensor("counts", (128, S), f32, kind="ExternalInput")
    ones_col_d = nc.dram_tensor("ones_col", (128, 1), f32, kind="ExternalInput")
    ones_row_d = nc.dram_tensor("ones_row", (1, 128), f32, kind="ExternalInput")
    outT_d = nc.dram_tensor("outT", (D, S), f32, kind="ExternalOutput")
    dbg_d = {}
    if debug:
        for n in ["avT", "z1", "z2", "ocr"]:
            dbg_d[n] = nc.dram_tensor("dbg_" + n, (D, S), f32,
                                      kind="ExternalOutput")

    with ExitStack() as ctx:
        tc = ctx.enter_context(tile.TileContext(nc))
        big = ctx.enter_context(tc.tile_pool(name="big", bufs=1))
        wk = ctx.enter_context(tc.tile_pool(name="wk", bufs=1))
        sm = ctx.enter_context(tc.tile_pool(name="sm", bufs=1))
        chain = ctx.enter_context(tc.tile_pool(name="chain", bufs=1))
        psp = ctx.enter_context(tc.tile_pool(name="psp", bufs=1, space="PSUM"))

        _ctr = [0]

        def mk(pool, shape, dtype, tag, bufs):
            _ctr[0] += 1
            return pool.tile(list(shape), dtype, tag=tag, bufs=bufs,
                             name=f"{tag}__{_ctr[0]}")

        # shared-slot makers
        def bb(dtype):  # persistent [128, S] activation tiles
            return mk(big, [128, S], dtype, "bb", 50)

        def pp():       # matmul accumulator banks
            return mk(psp, [128, S], f32, "pp", 4)

        def aux(p=128):  # other psum banks
            return mk(psp, [p, S], f32, "aux", 4)

        # ---------------- constants / biases ----------------
        L2_t = mk(big, [128, 128], f32r, "cL2", 1)
        nc.sync.dma_start(L2_t[:], L2_d[:].bitcast(f32r))
        counts_t = mk(big, [128, S], f32, "ccnt", 1)
        nc.sync.dma_start(counts_t[:], counts_d[:])
        ones_col_t = mk(big, [128, 1], f32r, "cones", 1)
        nc.sync.dma_start(ones_col_t[:], ones_col_d[:].bitcast(f32r))
        ones_row_t = mk(big, [1, 128], f32r, "conesr", 1)
        nc.sync.dma_start(ones_row_t[:], ones_row_d[:].bitcast(f32r))

        bcol = {}
        for n in BIAS_NAMES:
            t = mk(big, [128, NT], f32, "bias_" + n, 1)
            nc.sync.dma_start(t[:], b_d[n][:].rearrange("(j p) -> p j", p=128))
            bcol[n] = t

        def bias_slice(name, m):
            return bcol[name][:, m:m + 1]

        eps_col = mk(big, [1, 1], f32, "ceps", 1)
        nc.gpsimd.memset(eps_col[:], EPS)

        # ---------------- inputs ----------------
        def load_T(dram):
            ts = []
            for m in range(NT):
                t = bb(f32r)
                nc.sync.dma_start(
                    t[:], dram[m * 128:(m + 1) * 128, :].bitcast(f32r))
                ts.append(t)
            return ts

        xT = load_T(xT_d)
        hT = load_T(hT_d)

        # ---------------- generic projection ----------------
        def wrow_load(wname, half):
            """DMA the [1024, 512] half of W as 8 [128, 512] row tiles."""
            ts = []
            for k in range(NT):
                wt = mk(wk, [128, S], f32r, "w", 12)
                nc.sync.dma_start(
                    wt[:],
                    w_d[wname][k * 128:(k + 1) * 128,
                               half * S:(half + 1) * S].bitcast(f32r))
                ts.append(wt)
            return ts

        def proj(wname, rhs_tiles, consume):
            """out[m] = consume(m, sum_k W[kblk, mblk].T @ rhs[k]) for 8 m."""
            outs = []
            for half in range(2):
                wrows = wrow_load(wname, half)
                for mm in range(4):
                    m = half * 4 + mm
                    psum = pp()
                    for k in range(NT):
                        nc.tensor.matmul(
                            psum[:], wrows[k][:, mm * 128:(mm + 1) * 128],
                            rhs_tiles[k][:], start=(k == 0),
                            stop=(k == NT - 1))
                    outs.append(consume(m, psum))
            return outs

        def copy_out(dtype, scale, bias_name):
            def f(m, psum):
                t = bb(dtype)
                nc.scalar.activation(t[:], psum[:], AF.Identity,
                                     bias=bias_slice(bias_name, m), scale=scale)
                return t
            return f

        def resid_out(bias_name, other_tiles, dtype=f32r):
            def f(m, psum):
                t = bb(dtype)
                nc.vector.scalar_tensor_tensor(
                    t[:], psum[:], bias_slice(bias_name, m), other_tiles[m][:],
                    op0=OP.add, op1=OP.add)
                return t
            return f

        # ---------------- layernorm (transposed layout) ----------------
        def ln_row(dtype=f32):
            return mk(sm, [1, S], dtype, "ln_row", 3)

        def layer_norm(in_tiles, out_dtype=f32r):
            mu_ps = aux(1)
            s2_ps = aux(1)
            for m in range(NT):
                nc.tensor.matmul(mu_ps[:], ones_col_t[:], in_tiles[m][:],
                                 start=(m == 0), stop=(m == NT - 1))
            for m in range(NT):
                sq = mk(sm, [128, S], f32r, "ln_sq", 2)
                nc.scalar.activation(sq[:], in_tiles[m][:], AF.Square)
                nc.tensor.matmul(s2_ps[:], ones_col_t[:], sq[:],
                                 start=(m == 0), stop=(m == NT - 1))
            mu_row = ln_row(f32r)
            s2_row = ln_row()
            nc.scalar.activation(mu_row[:], mu_ps[:], AF.Copy, scale=1.0 / D)
            nc.scalar.activation(s2_row[:], s2_ps[:], AF.Copy, scale=1.0 / D)
            var_row = ln_row()
            # var = (mu * -1) * mu + s2
            nc.vector.scalar_tensor_tensor(
                var_row[:], mu_row[:], -1.0, mu_row[:],
                op0=OP.mult, op1=OP.mult)
            nc.vector.tensor_add(var_row[:], var_row[:], s2_row[:])
            lnv = ln_row()
            nc.scalar.activation(lnv[:], var_row[:], AF.Ln, bias=eps_col[:])
            rstd_row = ln_row(f32r)
            nc.scalar.activation(rstd_row[:], lnv[:], AF.Exp, scale=-0.5)
            mu_rep = aux()
            rs_rep = aux()
            nc.tensor.matmul(mu_rep[:], ones_row_t[:], mu_row[:],
                             start=True, stop=True)
            nc.tensor.matmul(rs_rep[:], ones_row_t[:], rstd_row[:],
                             start=True, stop=True)
            rs_rep_sb = mk(sm, [128, S], f32, "ln_rsrep", 2)
            nc.vector.tensor_copy(rs_rep_sb[:], rs_rep[:])
            outs = []
            for m in range(NT):
                diff = mk(sm, [128, S], f32, "ln_tmp", 2)
                nc.vector.tensor_sub(diff[:], in_tiles[m][:], mu_rep[:])
                g = mk(sm, [128, S], f32, "ln_tmp", 2)
                nc.vector.scalar_tensor_tensor(
                    g[:], diff[:], bias_slice("gamma", m), rs_rep_sb[:],
                    op0=OP.mult, op1=OP.mult)
                o = bb(out_dtype)
                nc.scalar.activation(o[:], g[:], AF.Identity,
                                     bias=bias_slice("beta", m))
                outs.append(o)
            return outs

        # ================= stage 1: self attention =================
        # QKV projections interleaved per m-tile so the tile-0 series can
        # start while PE continues projecting tiles 1..7 (phase overlap).
        A1, P1, G0 = [], [], []
        qkv_spec = [
            ("Wq", A1, copy_out(f32r, 1.0 / math.sqrt(C), "bq_s")),
            ("Wk", P1, copy_out(f32r, 1.0, "bk")),
            ("Wv", G0, copy_out(f32r, 1.0, "bv")),
        ]
        for half in range(2):
            for wname, lst, consume in qkv_spec:
                wrows = wrow_load(wname, half)
                for mm in range(4):
                    m = half * 4 + mm
                    psum = pp()
                    for k in range(NT):
                        nc.tensor.matmul(
                            psum[:], wrows[k][:, mm * 128:(mm + 1) * 128],
                            xT[k][:], start=(k == 0), stop=(k == NT - 1))
                    lst.append(consume(m, psum))

        def ch(dtype, tag, bufs=2):
            return mk(chain, [128, S], dtype, tag, bufs)

        KcT = proj("Wck", hT, copy_out(f32r, 1.0, "bck"))
        # bcv replicated across partitions (it varies along the free dim here)
        bcv_row = mk(sm, [1, HID], f32r, "bcv_row", 1)
        nc.sync.dma_start(
            bcv_row[:],
            b_d["bcv"][:].rearrange("(o d) -> o d", o=1).bitcast(f32r))
        bcv_rep = []
        for half in range(2):
            rep_ps = aux()
            nc.tensor.matmul(rep_ps[:], ones_row_t[:],
                             bcv_row[:, half * S:(half + 1) * S],
                             start=True, stop=True)
            rep_sb = mk(sm, [128, S], f32, "rep_sb", 2)
            nc.vector.tensor_copy(rep_sb[:], rep_ps[:])
            bcv_rep.append(rep_sb)
        # Vc natural [S, HID] in 4 row-tiles of [128, HID]
        Vc = [mk(big, [128, HID], f32r, "vc", 4) for _ in range(4)]
        for half in range(2):
            vps = [pp() for _ in range(4)]
            for k in range(NT):
                wt = mk(wk, [128, S], f32r, "w", 12)
                nc.sync.dma_start(
                    wt[:],
                    w_d["Wcv"][k * 128:(k + 1) * 128,
                               half * S:(half + 1) * S].bitcast(f32r))
                for kt in range(4):
                    nc.tensor.matmul(
                        vps[kt][:], hT[k][:, kt * 128:(kt + 1) * 128], wt[:],
                        start=(k == 0), stop=(k == NT - 1))
            for kt in range(4):
                nc.vector.tensor_add(Vc[kt][:, half * S:(half + 1) * S],
                                     vps[kt][:], bcv_rep[half][:])

        avT = []
        for i in range(NT):
            num = ch(f32, "num")
            den = ch(f32, "den")
            t_ps = aux()
            nc.tensor.matmul(t_ps[:], L2_t[:], G0[i][:], start=True, stop=True)
            nc.scalar.activation(num[:], t_ps[:], AF.Copy)
            An_prev, Pn_prev, Gn_prev = None, None, None
            for n in range(1, NTERMS):
                if n == 1:
                    An, Pn = A1[i], P1[i]
                    Gn = ch(f32r, "Gn")
                    nc.vector.tensor_mul(Gn[:], G0[i][:], P1[i][:])
                else:
                    An = ch(f32, "An")
                    nc.vector.scalar_tensor_tensor(
                        An[:], An_prev[:], 1.0 / n, A1[i][:],
                        op0=OP.mult, op1=OP.mult)
                    Pn = ch(f32r, "Pn")
                    nc.gpsimd.tensor_mul(Pn[:], Pn_prev[:], P1[i][:])
                    Gn = ch(f32r, "Gn")
                    geng = nc.gpsimd if n >= 5 else nc.vector
                    geng.tensor_mul(Gn[:], Gn_prev[:], P1[i][:])
                t_ps = aux()
                w_ps = aux()
                nc.tensor.matmul(t_ps[:], L2_t[:], Gn[:], start=True, stop=True)
                nc.tensor.matmul(w_ps[:], L2_t[:], Pn[:], start=True, stop=True)
                tmp = ch(f32, "tmp", 2)
                nc.vector.tensor_mul(tmp[:], An[:], t_ps[:])
                nc.gpsimd.tensor_add(num[:], num[:], tmp[:])
                if n == 1:
                    nc.vector.tensor_mul(den[:], An[:], w_ps[:])
                else:
                    tmp2 = ch(f32, "tmp2", 1)
                    nc.vector.tensor_mul(tmp2[:], An[:], w_ps[:])
                    nc.gpsimd.tensor_add(den[:], den[:], tmp2[:])
                An_prev, Pn_prev, Gn_prev = An, Pn, Gn
            nc.gpsimd.tensor_add(den[:], den[:], counts_t[:])
            lg = ch(f32, "tmp", 2)
            nc.scalar.activation(lg[:], den[:], AF.Ln)
            rec = ch(f32, "tmp", 2)
            nc.scalar.activation(rec[:], lg[:], AF.Exp, scale=-1.0)
            av = bb(f32r)
            nc.vector.tensor_mul(av[:], num[:], rec[:])
            avT.append(av)

        r1 = proj("Wo", avT, resid_out("bo", xT))
        z1 = layer_norm(r1)

        # ================= stage 2: cross attention =================
        QcT = proj("Wcq", z1, copy_out(f32r, 1.0, "bcq"))

        # per-head cross attention; softmax normalization folded into ES
        o_cr = [bb(f32r) for _ in range(NT)]
        for hd in range(NH):
            i, r0 = hd // 2, (hd % 2) * 64
            es_tiles = []
            for kt in range(4):
                s_ps = pp()
                nc.tensor.matmul(
                    s_ps[:], KcT[i][r0:r0 + 64, kt * 128:(kt + 1) * 128],
                    QcT[i][r0:r0 + 64, :], start=True, stop=True)
                es = mk(sm, [128, S], f32r, "cr_es", 4)
                nc.scalar.activation(es[:], s_ps[:], AF.Exp,
                                     scale=1.0 / math.sqrt(HID))
                es_tiles.append(es)
            d_ps = aux(1)
            for kt in range(4):
                nc.tensor.matmul(d_ps[:], ones_col_t[:], es_tiles[kt][:],
                                 start=(kt == 0), stop=(kt == 3))
            lg_row = mk(sm, [1, S], f32, "cr_row", 2)
            nc.scalar.activation(lg_row[:], d_ps[:], AF.Ln)
            rec_row = mk(sm, [1, S], f32r, "cr_row", 2)
            nc.scalar.activation(rec_row[:], lg_row[:], AF.Exp, scale=-1.0)
            rep_ps = aux()
            nc.tensor.matmul(rep_ps[:], ones_row_t[:], rec_row[:],
                             start=True, stop=True)
            for kt in range(4):
                nc.vector.tensor_mul(es_tiles[kt][:], es_tiles[kt][:],
                                     rep_ps[:])
            o_ps = aux(64)
            for kt in range(4):
                nc.tensor.matmul(o_ps[:], Vc[kt][:, hd * 64:(hd + 1) * 64],
                                 es_tiles[kt][:], start=(kt == 0),
                                 stop=(kt == 3))
            nc.scalar.activation(o_cr[i][r0:r0 + 64, :], o_ps[:], AF.Copy)

        r2 = proj("Wco", o_cr, resid_out("bco", z1))
        z2 = layer_norm(r2)

        # ================= stage 3: FFN =================
        u = proj("W1", z2, copy_out(f32r, 1.0, "b1"))
        r3 = proj("W2", u, resid_out("b2", z2))
        z3 = layer_norm(r3, out_dtype=f32)

        for m in range(NT):
            nc.sync.dma_start(outT_d[m * 128:(m + 1) * 128, :], z3[m][:])
        if debug:
            dbg_src = {"avT": avT, "z1": z1, "z2": z2, "ocr": o_cr}
            for n, tiles in dbg_src.items():
                for m in range(NT):
                    nc.sync.dma_start(
                        dbg_d[n][m * 128:(m + 1) * 128, :],
                        tiles[m][:].bitcast(f32))


def make_consts():
    L = np.zeros((128, 128), np.float32)
    for k in range(128):
        for q in range(128):
            if k // 64 == q // 64 and (k % 64) <= (q % 64):
                L[k, q] = 1.0
    counts = np.tile((np.arange(128, dtype=np.float32) % 64) + 1.0,
                     (S, 1)).T.copy()
    return {
        "L2": L,
        "counts": np.ascontiguousarray(counts),
        "ones_col": np.ones((128, 1), np.float32),
        "ones_row": np.ones((1, 128), np.float32),
    }


def make_in_maps(inputs):
    x = np.asarray(inputs["x"], np.float32)
    h = np.asarray(inputs["h"], np.float32)
    consts = make_consts()
    base = {n: np.ascontiguousarray(np.asarray(inputs[n], np.float32))
            for n in W_NAMES}
    biases = {"bq_s": np.asarray(inputs["bq"], np.float32) / math.sqrt(C)}
    for n in BIAS_NAMES[1:]:
        biases[n] = inputs[n]
    biases = {k: np.ascontiguousarray(np.asarray(v, np.float32))
              for k, v in biases.items()}
    in_maps = []
    for b in range(B):
        m = {"xT": np.ascontiguousarray(x[b].T),
             "hT": np.ascontiguousarray(h[b].T)}
        m.update(base)
        m.update(biases)
        m.update(consts)
        in_maps.append(m)
    return in_maps


_CACHE = {}


def get_program(debug=False):
    key = ("prog", debug)
    if key not in _CACHE:
        import concourse.bacc as bacc
        nc = bacc.Bacc(trn_type="TRN2")
        build(nc, debug=debug)
        nc.finalize()
        _CACHE[key] = nc
    return _CACHE[key]


def kernel(**inputs):
    from concourse.bass_utils import run_bass_kernel_spmd

    nc = get_program()
    in_maps = make_in_maps(inputs)
    res = run_bass_kernel_spmd(nc, in_maps, list(range(8)))
    out = np.stack([np.asarray(res.results[b]["outT"]).T for b in range(B)])
    return out.astype(np.float32)


if __name__ == "__main__":
    nc = get_program()
    print("built:", len(nc.inst_map), "instructions")
